# revision 16
# baseline (speedup 1.0000x reference)
"""Multi-head attention Trainium2 kernel (8 NeuronCores, SPMD).

Sharding: core c handles batch b = c//4 and the 4 heads [4*(c%4), 4*(c%4)+4).
Attention is fully independent per (batch, head); the output projection is
computed as per-core partial sums over the core's 256 ctx columns and reduced
on the host (plus bias).

Host pre-transposes x and the weight blocks so the device loads operands
directly in contraction-on-partition layout (no on-chip transposes).

Per-core device program (matmuls contract along the partition dim):
  qT/kT = W @ xT + b                        [dq, sq]  (head dim on partitions)
  v     = x @ WvT + bv                      [t, dv]   (natural layout)
  per sq-chunk of 1024, per head:
    sT   = scores transposed [t, sq] tiles (lhsT = kT slice, rhs = qT slice)
    A    = exp(sT/8)  (ACT, PSUM->SBUF, rounded to matmul dtype)
    ctxT + row-sums via [V | 1] ones-column matmul, contract over t
    reciprocal via PE row<->column transposes (128-lane DVE recip)
    normalize with PE-broadcast reciprocal row (DVE+GPSIMD); DMA A^T strips
  outT  = WoT @ ctxT per chunk (overlapped with next chunk), host-reduced

The attention tensor is produced transposed ([h, t, sq] per core); kernel()
returns a numpy transposed view so no host transpose copy is needed.

Env knobs:
  ATTN_MM_DTYPE = f32r (default) | f32   -- matmul operand dtype
  ATTN_GPS_TILES = 5                     -- normalize tiles per chunk on GPSIMD
"""

import os

import numpy as np

EMBED = 1024
NHEADS = 16
DH = 64
BSZ = 2
SEQ = 2048
NCORES = 8
HPC = 4          # heads per core
DQ = HPC * DH    # 256 projection columns per core

_MM_MODE = os.environ.get("ATTN_MM_DTYPE", "f32r")
_GPS_TILES = int(os.environ.get("ATTN_GPS_TILES", "5"))

_state = {}


def _build_nc():
    import concourse.bacc as bacc
    import concourse.mybir as mybir
    from concourse import masks
    from concourse.tile import TileContext

    F32 = mybir.dt.float32
    MMDT = mybir.dt.float32r if _MM_MODE == "f32r" else F32
    AF = mybir.ActivationFunctionType

    nc = bacc.Bacc(trn_type="TRN2", target_bir_lowering=False)

    XT = nc.declare_dram_parameter("xt", [EMBED, SEQ], F32, isOutput=False)
    WQT = nc.declare_dram_parameter("wqt", [EMBED, DQ], F32, isOutput=False)
    BQ = nc.declare_dram_parameter("bq", [DQ], F32, isOutput=False)
    WKT = nc.declare_dram_parameter("wkt", [EMBED, DQ], F32, isOutput=False)
    BK = nc.declare_dram_parameter("bk", [DQ], F32, isOutput=False)
    WVT = nc.declare_dram_parameter("wvt", [EMBED, DQ], F32, isOutput=False)
    BV = nc.declare_dram_parameter("bv", [DQ], F32, isOutput=False)
    WOT = nc.declare_dram_parameter("wot", [DQ, EMBED], F32, isOutput=False)
    ATT = nc.declare_dram_parameter("attnT", [HPC, SEQ, SEQ], F32, isOutput=True)
    OUTT = nc.declare_dram_parameter("outT", [EMBED, SEQ], F32, isOutput=True)

    with TileContext(nc) as tc:
        with (
            tc.tile_pool(name="const", bufs=1) as cp,
            tc.tile_pool(name="persist", bufs=1) as pp,
            tc.tile_pool(name="psum", bufs=1, space="PSUM") as psp,
        ):
            ident = cp.tile([128, 128], F32, tag="ident")
            masks.make_identity(nc, ident[:])
            ones1 = cp.tile([1, 128], F32, tag="ones1")
            nc.vector.memset(ones1[:], 1.0)
            ones4 = cp.tile([128, 4], F32, tag="ones4")
            nc.vector.memset(ones4[:], 1.0)
            bq_t = cp.tile([128, 2], F32, tag="bqt")
            nc.sync.dma_start(out=bq_t[:], in_=BQ.ap().rearrange("(j p) -> p j", p=128))
            bk_t = cp.tile([128, 2], F32, tag="bkt")
            nc.sync.dma_start(out=bk_t[:], in_=BK.ap().rearrange("(j p) -> p j", p=128))
            bv_row = cp.tile([1, DQ], F32, tag="bvr")
            nc.sync.dma_start(out=bv_row[:], in_=BV.ap().unsqueeze(0))
            warm = cp.tile([1, 2], F32, tag="warm")
            nc.scalar.activation(warm[:], bq_t[0:1, 0:2], AF.Exp)

            qT = [pp.tile([128, SEQ], MMDT, tag=f"qT{i}", name=f"qT{i}") for i in range(2)]
            kT = [pp.tile([128, SEQ], MMDT, tag=f"kT{i}", name=f"kT{i}") for i in range(2)]
            vsb = pp.tile([128, 16 * HPC * 65], MMDT, tag="vsb")  # [t-tile][head][64+ones]
            ctxT = [pp.tile([128, SEQ], MMDT, tag=f"ctxT{i}", name=f"ctxT{i}") for i in range(2)]
            woT = [pp.tile([128, 1024], MMDT, tag=f"woT{i}", name=f"woT{i}") for i in range(2)]

            for j in range(2):
                nc.gpsimd.dma_start(out=woT[j][:], in_=WOT.ap()[128 * j:128 * (j + 1), :])

            # ---------- phase 1: projections ----------
            with tc.tile_pool(name="ph1sb", bufs=1) as xp:
                xT = xp.tile([128, 8 * SEQ], MMDT, tag="xT")
                wqT = xp.tile([128, 8 * DQ], MMDT, tag="wqT")
                wkT = xp.tile([128, 8 * DQ], MMDT, tag="wkT")
                wvT = xp.tile([128, 8 * DQ], MMDT, tag="wvT")

                xtv = XT.ap().rearrange("(j p) s -> p j s", p=128)
                for j in range(8):
                    nc.gpsimd.dma_start(
                        out=xT[:, SEQ * j:SEQ * (j + 1)], in_=xtv[:, j, :]
                    )
                for W, wT in ((WQT, wqT), (WKT, wkT), (WVT, wvT)):
                    wv_ = W.ap().rearrange("(j p) m -> p j m", p=128)
                    for g in range(2):
                        nc.gpsimd.dma_start(
                            out=wT[:, 1024 * g:1024 * (g + 1)], in_=wv_[:, 4 * g:4 * (g + 1), :]
                        )

                # qT / kT: lhsT (weight slice) reused across 4 n-chunks;
                # two [128,1024] psum tiles hold the 4 chunks
                for wT, qk, bt in ((wqT, qT, bq_t), (wkT, kT, bk_t)):
                    for mi in range(2):
                        pq = [psp.tile([128, 1024], F32, tag="score", name=f"pq{mi}_{u}")
                              for u in range(2)]
                        for j in range(8):
                            for n in range(4):
                                nc.tensor.matmul(
                                    pq[n // 2][:, 512 * (n % 2):512 * (n % 2 + 1)],
                                    wT[:, 256 * j + 128 * mi:256 * j + 128 * (mi + 1)],
                                    xT[:, SEQ * j + 512 * n:SEQ * j + 512 * (n + 1)],
                                    start=(j == 0),
                                    stop=(j == 7),
                                )
                        for u in range(2):
                            nc.scalar.activation(
                                qk[mi][:, 1024 * u:1024 * (u + 1)],
                                pq[u][:],
                                AF.Identity,
                                bias=bt[:, mi:mi + 1],
                                scale=1.0,
                            )

                # v projection into [V | 1] layout (ones cols via ACT casts)
                vview = vsb[:].rearrange("p (t h e) -> p t h e", t=16, h=HPC)
                for tt in range(16):
                    nc.scalar.copy(out=vview[:, tt, :, 64:65], in_=ones4[:].unsqueeze(-1))
                    pv = psp.tile([128, DQ], F32, tag="score", name=f"pv{tt}")
                    for j in range(8):
                        nc.tensor.matmul(
                            pv[:],
                            xT[:, SEQ * j + 128 * tt:SEQ * j + 128 * (tt + 1)],
                            wvT[:, 256 * j:256 * (j + 1)],
                            start=(j == 0),
                            stop=False,
                        )
                    nc.tensor.matmul(pv[:], ones1[:], bv_row[:], start=False, stop=True)
                    nc.scalar.copy(
                        out=vview[:, tt, :, 0:64],
                        in_=pv[:].rearrange("p (h d) -> p h d", h=HPC),
                    )

            # ---------- phase 2: attention (chunk-outer) + overlapped out-proj ----------
            with (
                tc.tile_pool(name="strips", bufs=5) as sp,
                tc.tile_pool(name="ostrips", bufs=2) as op2,
                tc.tile_pool(name="rp", bufs=1) as rp,
                tc.tile_pool(name="bp", bufs=2) as bp,
                tc.tile_pool(name="osb", bufs=2) as op,
            ):
                for ch in range(2):
                    sq0 = 1024 * ch
                    for h in range(HPC):
                        mi, po = h // 2, 64 * (h % 2)
                        att_h = ATT.ap()[h].rearrange("(t p) s -> p t s", p=128)
                        quarters = [sp.tile([128, 4 * 1024], MMDT, tag="strip",
                                            name=f"strip{h}_{ch}_{i}") for i in range(4)]
                        pc = psp.tile([65, 1024], F32, tag="ctx", name=f"pc{h}_{ch}")
                        for tt in range(16):
                            qs, tl = tt // 4, tt % 4
                            ps = psp.tile([128, 1024], F32, tag="score", name=f"ps{h}_{ch}_{tt}")
                            for n in range(2):
                                nc.tensor.matmul(
                                    ps[:, 512 * n:512 * (n + 1)],
                                    kT[mi][po:po + 64, 128 * tt:128 * (tt + 1)],
                                    qT[mi][po:po + 64, sq0 + 512 * n:sq0 + 512 * (n + 1)],
                                    start=True,
                                    stop=True,
                                )
                            nc.scalar.activation(
                                quarters[qs][:, 1024 * tl:1024 * (tl + 1)],
                                ps[:],
                                AF.Exp,
                                scale=0.125,
                            )
                            for n in range(2):
                                nc.tensor.matmul(
                                    pc[:, 512 * n:512 * (n + 1)],
                                    vview[:, tt, h, :],
                                    quarters[qs][:, 1024 * tl + 512 * n:1024 * tl + 512 * (n + 1)],
                                    start=(tt == 0),
                                    stop=(tt == 15),
                                )
                        # --- reciprocal of row-sums on 128 lanes via PE transposes ---
                        zrow = rp.tile([1, 1024], F32, tag="zrow", name=f"zr{h}_{ch}")
                        nc.scalar.copy(out=zrow[:], in_=pc[64:65, :])
                        pzt = psp.tile([128, 1024], F32, tag="bcast", name=f"pzt{h}_{ch}")
                        for g in range(8):
                            nc.tensor.transpose(
                                pzt[:, g:g + 1], zrow[0:1, 128 * g:128 * (g + 1)],
                                ones1[0:1, 0:1],
                            )
                        zt = rp.tile([128, 8], F32, tag="zt", name=f"zt{h}_{ch}")
                        nc.scalar.copy(out=zt[:], in_=pzt[:, 0:8])
                        rt = rp.tile([128, 8], F32, tag="rt", name=f"rt{h}_{ch}")
                        nc.vector.reciprocal(rt[:], zt[:])
                        prr = psp.tile([128, 1024], F32, tag="bcast", name=f"prr{h}_{ch}")
                        for g in range(8):
                            nc.tensor.transpose(
                                prr[0:1, 128 * g:128 * (g + 1)], rt[:, g:g + 1], ident[:]
                            )
                        rrow = rp.tile([1, 1024], F32, tag="rrow", name=f"rr{h}_{ch}")
                        nc.scalar.copy(out=rrow[:], in_=prr[0:1, :])
                        pb = psp.tile([128, 1024], F32, tag="bcast", name=f"pb{h}_{ch}")
                        for n in range(2):
                            nc.tensor.matmul(
                                pb[:, 512 * n:512 * (n + 1)],
                                ones1[:],
                                rrow[0:1, 512 * n:512 * (n + 1)],
                                start=True,
                                stop=True,
                            )
                        bcast = bp.tile([128, 1024], F32, tag="bcast_sb", name=f"bc{h}_{ch}")
                        nc.vector.tensor_copy(out=bcast[:], in_=pb[:])
                        # ctx normalize -> ctxT
                        ctmp = rp.tile([64, 1024], F32, tag="ctmp", name=f"ctmp{h}_{ch}")
                        nc.scalar.copy(out=ctmp[:], in_=pc[0:64, :])
                        nc.vector.tensor_mul(
                            ctxT[mi][po:po + 64, sq0:sq0 + 1024], ctmp[:], bcast[0:64, :]
                        )
                        # normalize A^T into f32 out-strips + DMA (DVE + GPSIMD split)
                        for tt in range(16):
                            qs, tl = tt // 4, tt % 4
                            ostrip = op2.tile([128, 1024], F32, tag="ostrip",
                                              name=f"os{h}_{ch}_{tt}")
                            eng = nc.gpsimd if tt < _GPS_TILES else nc.vector
                            eng.tensor_mul(
                                ostrip[:],
                                quarters[qs][:, 1024 * tl:1024 * (tl + 1)].bitcast(F32),
                                bcast[:],
                            )
                            nc.sync.dma_start(
                                out=att_h[:, tt, sq0:sq0 + 1024],
                                in_=ostrip[:],
                            )
                    # out-projection for this chunk's columns (overlaps next chunk)
                    for dt_ in range(8):
                        po_ = psp.tile([128, 1024], F32, tag="score", name=f"po{ch}_{dt_}")
                        for it in range(2):
                            for nn in range(2):
                                nc.tensor.matmul(
                                    po_[:, 512 * nn:512 * (nn + 1)],
                                    woT[it][:, 128 * dt_:128 * (dt_ + 1)],
                                    ctxT[it][:, sq0 + 512 * nn:sq0 + 512 * (nn + 1)],
                                    start=(it == 0),
                                    stop=(it == 1),
                                )
                        osb = op.tile([128, 1024], F32, tag="outT", name=f"osb{ch}_{dt_}")
                        nc.scalar.copy(out=osb[:], in_=po_[:])
                        nc.sync.dma_start(
                            out=OUTT.ap()[128 * dt_:128 * (dt_ + 1), sq0:sq0 + 1024],
                            in_=osb[:],
                        )

    nc.finalize()
    return nc


def _get_nc():
    if "nc" not in _state:
        _state["nc"] = _build_nc()
    return _state["nc"]


def _shard_inputs(x, Wq, bq, Wk, bk, Wv, bv, Wo, bo):
    f = lambda a: np.ascontiguousarray(np.asarray(a, dtype=np.float32))
    x, Wq, bq, Wk, bk, Wv, bv, Wo, bo = map(f, (x, Wq, bq, Wk, bk, Wv, bv, Wo, bo))
    xT = [np.ascontiguousarray(x[b].T) for b in range(BSZ)]            # [E, S]
    WqT, WkT, WvT = Wq.T, Wk.T, Wv.T                                   # [E, DQall]
    WoT = np.ascontiguousarray(Wo.T)                                   # [DQall, E]
    in_maps = []
    for c in range(NCORES):
        b, hb = c // 4, c % 4
        sl = slice(DQ * hb, DQ * (hb + 1))
        in_maps.append({
            "xt": xT[b],
            "wqt": np.ascontiguousarray(WqT[:, sl]), "bq": np.ascontiguousarray(bq[sl]),
            "wkt": np.ascontiguousarray(WkT[:, sl]), "bk": np.ascontiguousarray(bk[sl]),
            "wvt": np.ascontiguousarray(WvT[:, sl]), "bv": np.ascontiguousarray(bv[sl]),
            "wot": np.ascontiguousarray(WoT[sl, :]),
        })
    return in_maps, bo


def kernel(x, Wq, bq, Wk, bk, Wv, bv, Wo, bo):
    from concourse.bass_utils import run_bass_kernel_spmd

    nc = _get_nc()
    in_maps, bo_np = _shard_inputs(x, Wq, bq, Wk, bk, Wv, bv, Wo, bo)
    res = run_bass_kernel_spmd(nc, in_maps, core_ids=list(range(NCORES)))
    _state["last"] = res

    attnT = np.empty((BSZ, NHEADS, SEQ, SEQ), np.float32)
    out = np.zeros((BSZ, SEQ, EMBED), np.float32)
    for c in range(NCORES):
        b, hb = c // 4, c % 4
        r = res.results[c]
        attnT[b, HPC * hb:HPC * (hb + 1)] = r["attnT"]
        out[b] += r["outT"].T
    out += bo_np
    return out, attnT.swapaxes(2, 3)


# revision 18
# speedup vs baseline: 1.0214x; 1.0214x over previous
"""Multi-head attention Trainium2 kernel (8 NeuronCores, SPMD).

Sharding: core c handles batch b = c//4 and the 4 heads [4*(c%4), 4*(c%4)+4).
Attention is fully independent per (batch, head); the output projection is
computed as per-core partial sums over the core's 256 ctx columns and reduced
on the host (plus bias).

Host pre-transposes x and the weight blocks so the device loads operands
directly in contraction-on-partition layout (no on-chip transposes).

Per-core device program (matmuls contract along the partition dim):
  qT/kT = W @ xT + b                        [dq, sq]  (head dim on partitions)
  v     = x @ WvT + bv                      [t, dv]   (natural layout)
  per sq-chunk of 1024, per head:
    sT   = scores transposed [t, sq] tiles (lhsT = kT slice, rhs = qT slice)
    A    = exp(sT/8)  (ACT, PSUM->SBUF, rounded to matmul dtype)
    ctxT + row-sums via [V | 1] ones-column matmul, contract over t
    reciprocal via PE row<->column transposes (128-lane DVE recip)
    normalize with PE-broadcast reciprocal row (DVE+GPSIMD); DMA A^T strips
  outT  = WoT @ ctxT per chunk (overlapped with next chunk), host-reduced

The attention tensor is produced transposed ([h, t, sq] per core); kernel()
returns a numpy transposed view so no host transpose copy is needed.

Env knobs:
  ATTN_MM_DTYPE = f32r (default) | f32   -- matmul operand dtype
  ATTN_GPS_TILES = 5                     -- normalize tiles per chunk on GPSIMD
"""

import os

import numpy as np

EMBED = 1024
NHEADS = 16
DH = 64
BSZ = 2
SEQ = 2048
NCORES = 8
HPC = 4          # heads per core
DQ = HPC * DH    # 256 projection columns per core

_MM_MODE = os.environ.get("ATTN_MM_DTYPE", "f32r")
_GPS_TILES = int(os.environ.get("ATTN_GPS_TILES", "4"))

_state = {}


def _build_nc():
    import concourse.bacc as bacc
    import concourse.mybir as mybir
    from concourse import masks
    from concourse.tile import TileContext

    F32 = mybir.dt.float32
    MMDT = mybir.dt.float32r if _MM_MODE == "f32r" else F32
    AF = mybir.ActivationFunctionType

    nc = bacc.Bacc(trn_type="TRN2", target_bir_lowering=False)

    XT = nc.declare_dram_parameter("xt", [EMBED, SEQ], F32, isOutput=False)
    WQT = nc.declare_dram_parameter("wqt", [EMBED, DQ], F32, isOutput=False)
    BQ = nc.declare_dram_parameter("bq", [DQ], F32, isOutput=False)
    WKT = nc.declare_dram_parameter("wkt", [EMBED, DQ], F32, isOutput=False)
    BK = nc.declare_dram_parameter("bk", [DQ], F32, isOutput=False)
    WVT = nc.declare_dram_parameter("wvt", [EMBED, DQ], F32, isOutput=False)
    BV = nc.declare_dram_parameter("bv", [DQ], F32, isOutput=False)
    WOT = nc.declare_dram_parameter("wot", [DQ, EMBED], F32, isOutput=False)
    ATT = nc.declare_dram_parameter("attnT", [HPC, SEQ, SEQ], F32, isOutput=True)
    OUTT = nc.declare_dram_parameter("outT", [EMBED, SEQ], F32, isOutput=True)

    with TileContext(nc) as tc:
        with (
            tc.tile_pool(name="const", bufs=1) as cp,
            tc.tile_pool(name="persist", bufs=1) as pp,
            tc.tile_pool(name="psum", bufs=1, space="PSUM") as psp,
        ):
            ident = cp.tile([128, 128], F32, tag="ident")
            masks.make_identity(nc, ident[:])
            ones1 = cp.tile([1, 128], F32, tag="ones1")
            nc.vector.memset(ones1[:], 1.0)
            ones4 = cp.tile([128, 4], F32, tag="ones4")
            nc.vector.memset(ones4[:], 1.0)
            bq_t = cp.tile([128, 2], F32, tag="bqt")
            nc.sync.dma_start(out=bq_t[:], in_=BQ.ap().rearrange("(j p) -> p j", p=128))
            bk_t = cp.tile([128, 2], F32, tag="bkt")
            nc.sync.dma_start(out=bk_t[:], in_=BK.ap().rearrange("(j p) -> p j", p=128))
            bv_row = cp.tile([1, DQ], F32, tag="bvr")
            nc.sync.dma_start(out=bv_row[:], in_=BV.ap().unsqueeze(0))
            warm = cp.tile([1, 2], F32, tag="warm")
            nc.scalar.activation(warm[:], bq_t[0:1, 0:2], AF.Exp)

            qT = [pp.tile([128, SEQ], MMDT, tag=f"qT{i}", name=f"qT{i}") for i in range(2)]
            kT = [pp.tile([128, SEQ], MMDT, tag=f"kT{i}", name=f"kT{i}") for i in range(2)]
            vsb = pp.tile([128, 16 * HPC * 65], MMDT, tag="vsb")  # [t-tile][head][64+ones]
            ctxT = [pp.tile([128, SEQ], MMDT, tag=f"ctxT{i}", name=f"ctxT{i}") for i in range(2)]
            woT = [pp.tile([128, 1024], MMDT, tag=f"woT{i}", name=f"woT{i}") for i in range(2)]

            for j in range(2):
                nc.gpsimd.dma_start(out=woT[j][:], in_=WOT.ap()[128 * j:128 * (j + 1), :])

            # ---------- phase 1: projections ----------
            with tc.tile_pool(name="ph1sb", bufs=1) as xp:
                xT = xp.tile([128, 8 * SEQ], MMDT, tag="xT")
                wqT = xp.tile([128, 8 * DQ], MMDT, tag="wqT")
                wkT = xp.tile([128, 8 * DQ], MMDT, tag="wkT")
                wvT = xp.tile([128, 8 * DQ], MMDT, tag="wvT")

                xtv = XT.ap().rearrange("(j p) s -> p j s", p=128)
                for j in range(8):
                    nc.gpsimd.dma_start(
                        out=xT[:, SEQ * j:SEQ * (j + 1)], in_=xtv[:, j, :]
                    )
                for W, wT in ((WQT, wqT), (WKT, wkT), (WVT, wvT)):
                    wv_ = W.ap().rearrange("(j p) m -> p j m", p=128)
                    for g in range(2):
                        nc.gpsimd.dma_start(
                            out=wT[:, 1024 * g:1024 * (g + 1)], in_=wv_[:, 4 * g:4 * (g + 1), :]
                        )

                # qT / kT: lhsT (weight slice) reused across 4 n-chunks;
                # two [128,1024] psum tiles hold the 4 chunks
                for wT, qk, bt in ((wqT, qT, bq_t), (wkT, kT, bk_t)):
                    for mi in range(2):
                        pq = [psp.tile([128, 1024], F32, tag="score", name=f"pq{mi}_{u}")
                              for u in range(2)]
                        for j in range(8):
                            for n in range(4):
                                nc.tensor.matmul(
                                    pq[n // 2][:, 512 * (n % 2):512 * (n % 2 + 1)],
                                    wT[:, 256 * j + 128 * mi:256 * j + 128 * (mi + 1)],
                                    xT[:, SEQ * j + 512 * n:SEQ * j + 512 * (n + 1)],
                                    start=(j == 0),
                                    stop=(j == 7),
                                )
                        for u in range(2):
                            nc.scalar.activation(
                                qk[mi][:, 1024 * u:1024 * (u + 1)],
                                pq[u][:],
                                AF.Identity,
                                bias=bt[:, mi:mi + 1],
                                scale=1.0,
                            )

                # v projection into [V | 1] layout (ones cols via ACT casts)
                vview = vsb[:].rearrange("p (t h e) -> p t h e", t=16, h=HPC)
                for tt in range(16):
                    nc.scalar.copy(out=vview[:, tt, :, 64:65], in_=ones4[:].unsqueeze(-1))
                    pv = psp.tile([128, DQ], F32, tag="score", name=f"pv{tt}")
                    for j in range(8):
                        nc.tensor.matmul(
                            pv[:],
                            xT[:, SEQ * j + 128 * tt:SEQ * j + 128 * (tt + 1)],
                            wvT[:, 256 * j:256 * (j + 1)],
                            start=(j == 0),
                            stop=False,
                        )
                    nc.tensor.matmul(pv[:], ones1[:], bv_row[:], start=False, stop=True)
                    nc.scalar.copy(
                        out=vview[:, tt, :, 0:64],
                        in_=pv[:].rearrange("p (h d) -> p h d", h=HPC),
                    )

            # ---------- phase 2: attention (chunk-outer) + overlapped out-proj ----------
            with (
                tc.tile_pool(name="strips", bufs=3) as sp,
                tc.tile_pool(name="ostrips", bufs=2) as op2,
                tc.tile_pool(name="rp", bufs=1) as rp,
                tc.tile_pool(name="bp", bufs=2) as bp,
                tc.tile_pool(name="osb", bufs=1) as op,
            ):
                for ch in range(2):
                    sq0 = 1024 * ch
                    for h in range(HPC):
                        mi, po = h // 2, 64 * (h % 2)
                        att_h = ATT.ap()[h].rearrange("(t p) s -> p t s", p=128)
                        halves = [sp.tile([128, 8 * 1024], MMDT, tag="strip",
                                          name=f"strip{h}_{ch}_{i}") for i in range(2)]
                        pc = psp.tile([65, 1024], F32, tag="ctx", name=f"pc{h}_{ch}")
                        for tt in range(16):
                            qs, tl = tt // 8, tt % 8
                            ps = psp.tile([128, 1024], F32, tag="score", name=f"ps{h}_{ch}_{tt}")
                            for n in range(2):
                                nc.tensor.matmul(
                                    ps[:, 512 * n:512 * (n + 1)],
                                    kT[mi][po:po + 64, 128 * tt:128 * (tt + 1)],
                                    qT[mi][po:po + 64, sq0 + 512 * n:sq0 + 512 * (n + 1)],
                                    start=True,
                                    stop=True,
                                )
                            nc.scalar.activation(
                                halves[qs][:, 1024 * tl:1024 * (tl + 1)],
                                ps[:],
                                AF.Exp,
                                scale=0.125,
                            )
                            for n in range(2):
                                nc.tensor.matmul(
                                    pc[:, 512 * n:512 * (n + 1)],
                                    vview[:, tt, h, :],
                                    halves[qs][:, 1024 * tl + 512 * n:1024 * tl + 512 * (n + 1)],
                                    start=(tt == 0),
                                    stop=(tt == 15),
                                )
                        # --- reciprocal of row-sums on 128 lanes via PE transposes ---
                        zrow = rp.tile([1, 1024], F32, tag="zrow", name=f"zr{h}_{ch}")
                        nc.scalar.copy(out=zrow[:], in_=pc[64:65, :])
                        pzt = psp.tile([128, 1024], F32, tag="bcast", name=f"pzt{h}_{ch}")
                        for g in range(8):
                            nc.tensor.transpose(
                                pzt[:, g:g + 1], zrow[0:1, 128 * g:128 * (g + 1)],
                                ones1[0:1, 0:1],
                            )
                        rt = rp.tile([128, 8], F32, tag="rt", name=f"rt{h}_{ch}")
                        nc.vector.reciprocal(rt[:], pzt[:, 0:8])
                        prr = psp.tile([128, 1024], F32, tag="bcast", name=f"prr{h}_{ch}")
                        for g in range(8):
                            nc.tensor.transpose(
                                prr[0:1, 128 * g:128 * (g + 1)], rt[:, g:g + 1], ident[:]
                            )
                        rrow = rp.tile([1, 1024], F32, tag="zrow", name=f"rr{h}_{ch}")
                        nc.scalar.copy(out=rrow[:], in_=prr[0:1, :])
                        pb = psp.tile([128, 1024], F32, tag="bcast", name=f"pb{h}_{ch}")
                        for n in range(2):
                            nc.tensor.matmul(
                                pb[:, 512 * n:512 * (n + 1)],
                                ones1[:],
                                rrow[0:1, 512 * n:512 * (n + 1)],
                                start=True,
                                stop=True,
                            )
                        bcast = bp.tile([128, 1024], F32, tag="bcast_sb", name=f"bc{h}_{ch}")
                        nc.vector.tensor_copy(out=bcast[:], in_=pb[:])
                        # normalize A^T into f32 out-strips + DMA (DVE + GPSIMD split)
                        for e in range(8):
                            hs, t0 = e // 4, (e % 4) * 2
                            ostrip = op2.tile([128, 2048], F32, tag="ostrip",
                                              name=f"os{h}_{ch}_{e}")
                            for u in range(2):
                                eng = nc.gpsimd if (2 * e + u) < _GPS_TILES else nc.vector
                                eng.tensor_mul(
                                    ostrip[:, 1024 * u:1024 * (u + 1)],
                                    halves[hs][:, 1024 * (t0 + u):1024 * (t0 + u + 1)].bitcast(F32),
                                    bcast[:],
                                )
                            nc.sync.dma_start(
                                out=att_h[:, 8 * hs + t0:8 * hs + t0 + 2, sq0:sq0 + 1024],
                                in_=ostrip[:].rearrange("p (t s) -> p t s", t=2),
                            )
                        # ctx normalize -> ctxT (PSUM in0 + SBUF in1 is legal)
                        nc.vector.tensor_mul(
                            ctxT[mi][po:po + 64, sq0:sq0 + 1024], pc[0:64, :], bcast[0:64, :]
                        )
                    # out-projection for this chunk's columns (overlaps next chunk)
                    for dt_ in range(8):
                        po_ = psp.tile([128, 1024], F32, tag="bcast", name=f"po{ch}_{dt_}")
                        for it in range(2):
                            for nn in range(2):
                                nc.tensor.matmul(
                                    po_[:, 512 * nn:512 * (nn + 1)],
                                    woT[it][:, 128 * dt_:128 * (dt_ + 1)],
                                    ctxT[it][:, sq0 + 512 * nn:sq0 + 512 * (nn + 1)],
                                    start=(it == 0),
                                    stop=(it == 1),
                                )
                        osb = op.tile([128, 1024], F32, tag="outT", name=f"osb{ch}_{dt_}")
                        nc.scalar.copy(out=osb[:], in_=po_[:])
                        nc.sync.dma_start(
                            out=OUTT.ap()[128 * dt_:128 * (dt_ + 1), sq0:sq0 + 1024],
                            in_=osb[:],
                        )

    nc.finalize()
    return nc


def _get_nc():
    if "nc" not in _state:
        _state["nc"] = _build_nc()
    return _state["nc"]


def _shard_inputs(x, Wq, bq, Wk, bk, Wv, bv, Wo, bo):
    f = lambda a: np.ascontiguousarray(np.asarray(a, dtype=np.float32))
    x, Wq, bq, Wk, bk, Wv, bv, Wo, bo = map(f, (x, Wq, bq, Wk, bk, Wv, bv, Wo, bo))
    xT = [np.ascontiguousarray(x[b].T) for b in range(BSZ)]            # [E, S]
    WqT, WkT, WvT = Wq.T, Wk.T, Wv.T                                   # [E, DQall]
    WoT = np.ascontiguousarray(Wo.T)                                   # [DQall, E]
    in_maps = []
    for c in range(NCORES):
        b, hb = c // 4, c % 4
        sl = slice(DQ * hb, DQ * (hb + 1))
        in_maps.append({
            "xt": xT[b],
            "wqt": np.ascontiguousarray(WqT[:, sl]), "bq": np.ascontiguousarray(bq[sl]),
            "wkt": np.ascontiguousarray(WkT[:, sl]), "bk": np.ascontiguousarray(bk[sl]),
            "wvt": np.ascontiguousarray(WvT[:, sl]), "bv": np.ascontiguousarray(bv[sl]),
            "wot": np.ascontiguousarray(WoT[sl, :]),
        })
    return in_maps, bo


def kernel(x, Wq, bq, Wk, bk, Wv, bv, Wo, bo):
    from concourse.bass_utils import run_bass_kernel_spmd

    nc = _get_nc()
    in_maps, bo_np = _shard_inputs(x, Wq, bq, Wk, bk, Wv, bv, Wo, bo)
    res = run_bass_kernel_spmd(nc, in_maps, core_ids=list(range(NCORES)))
    _state["last"] = res

    attnT = np.empty((BSZ, NHEADS, SEQ, SEQ), np.float32)
    out = np.zeros((BSZ, SEQ, EMBED), np.float32)
    for c in range(NCORES):
        b, hb = c // 4, c % 4
        r = res.results[c]
        attnT[b, HPC * hb:HPC * (hb + 1)] = r["attnT"]
        out[b] += r["outT"].T
    out += bo_np
    return out, attnT.swapaxes(2, 3)


# revision 19
# speedup vs baseline: 1.1077x; 1.0845x over previous
"""Multi-head attention Trainium2 kernel (8 NeuronCores, SPMD).

Sharding: core c handles batch b = c//4 and the 4 heads [4*(c%4), 4*(c%4)+4).
Attention is fully independent per (batch, head); the output projection is
computed as per-core partial sums over the core's 256 ctx columns and reduced
on the host (plus bias).

Host pre-transposes x and the weight blocks so the device loads operands
directly in contraction-on-partition layout (no on-chip transposes).

Per-core device program (matmuls contract along the partition dim):
  qT/kT = W @ xT + b                        [dq, sq]  (head dim on partitions)
  v     = x @ WvT + bv                      [t, dv]   (natural layout)
  per sq-chunk of 1024, per head:
    sT   = scores transposed [t, sq] tiles (lhsT = kT slice, rhs = qT slice)
    A    = exp(sT/8)  (ACT, PSUM->SBUF, rounded to matmul dtype)
    ctxT + row-sums via [V | 1] ones-column matmul, contract over t
    reciprocal via PE row<->column transposes (128-lane DVE recip)
    normalize with PE-broadcast reciprocal row (DVE+GPSIMD); DMA A^T strips
  outT  = WoT @ ctxT per chunk (overlapped with next chunk), host-reduced

The attention tensor is produced transposed ([h, t, sq] per core); kernel()
returns a numpy transposed view so no host transpose copy is needed.

Env knobs:
  ATTN_MM_DTYPE = f32r (default) | f32   -- matmul operand dtype
  ATTN_GPS_TILES = 5                     -- normalize tiles per chunk on GPSIMD
"""

import os

import numpy as np

EMBED = 1024
NHEADS = 16
DH = 64
BSZ = 2
SEQ = 2048
NCORES = 8
HPC = 4          # heads per core
DQ = HPC * DH    # 256 projection columns per core

_MM_MODE = os.environ.get("ATTN_MM_DTYPE", "f32r")
_GPS_TILES = int(os.environ.get("ATTN_GPS_TILES", "4"))

_state = {}


def _build_nc():
    import concourse.bacc as bacc
    import concourse.mybir as mybir
    from concourse import masks
    from concourse.tile import TileContext

    F32 = mybir.dt.float32
    MMDT = mybir.dt.float32r if _MM_MODE == "f32r" else F32
    AF = mybir.ActivationFunctionType

    nc = bacc.Bacc(trn_type="TRN2", target_bir_lowering=False)

    XT = nc.declare_dram_parameter("xt", [EMBED, SEQ], F32, isOutput=False)
    WQT = nc.declare_dram_parameter("wqt", [EMBED, DQ], F32, isOutput=False)
    BQ = nc.declare_dram_parameter("bq", [DQ], F32, isOutput=False)
    WKT = nc.declare_dram_parameter("wkt", [EMBED, DQ], F32, isOutput=False)
    BK = nc.declare_dram_parameter("bk", [DQ], F32, isOutput=False)
    WVT = nc.declare_dram_parameter("wvt", [EMBED, DQ], F32, isOutput=False)
    BV = nc.declare_dram_parameter("bv", [DQ], F32, isOutput=False)
    WOT = nc.declare_dram_parameter("wot", [DQ, EMBED], F32, isOutput=False)
    ATT = nc.declare_dram_parameter("attnT", [HPC, SEQ, SEQ], F32, isOutput=True)
    OUTT = nc.declare_dram_parameter("outT", [EMBED, SEQ], F32, isOutput=True)

    with TileContext(nc) as tc:
        with (
            tc.tile_pool(name="const", bufs=1) as cp,
            tc.tile_pool(name="persist", bufs=1) as pp,
            tc.tile_pool(name="psum", bufs=1, space="PSUM") as psp,
        ):
            ident = cp.tile([128, 128], F32, tag="ident")
            masks.make_identity(nc, ident[:])
            ones1 = cp.tile([1, 128], F32, tag="ones1")
            nc.vector.memset(ones1[:], 1.0)
            ones4 = cp.tile([128, 4], F32, tag="ones4")
            nc.vector.memset(ones4[:], 1.0)
            bq_t = cp.tile([128, 2], F32, tag="bqt")
            nc.sync.dma_start(out=bq_t[:], in_=BQ.ap().rearrange("(j p) -> p j", p=128))
            bk_t = cp.tile([128, 2], F32, tag="bkt")
            nc.sync.dma_start(out=bk_t[:], in_=BK.ap().rearrange("(j p) -> p j", p=128))
            bv_row = cp.tile([1, DQ], F32, tag="bvr")
            nc.sync.dma_start(out=bv_row[:], in_=BV.ap().unsqueeze(0))
            warm = cp.tile([1, 2], F32, tag="warm")
            nc.scalar.activation(warm[:], bq_t[0:1, 0:2], AF.Exp)

            qT = [pp.tile([128, SEQ], MMDT, tag=f"qT{i}", name=f"qT{i}") for i in range(2)]
            kT = [pp.tile([128, SEQ], MMDT, tag=f"kT{i}", name=f"kT{i}") for i in range(2)]
            vsb = pp.tile([128, 16 * HPC * 65], MMDT, tag="vsb")  # [t-tile][head][64+ones]
            ctxT = [pp.tile([128, SEQ], MMDT, tag=f"ctxT{i}", name=f"ctxT{i}") for i in range(2)]
            woT = [pp.tile([128, 1024], MMDT, tag=f"woT{i}", name=f"woT{i}") for i in range(2)]

            for j in range(2):
                nc.gpsimd.dma_start(out=woT[j][:], in_=WOT.ap()[128 * j:128 * (j + 1), :])

            # ---------- phase 1: projections ----------
            with tc.tile_pool(name="ph1sb", bufs=1) as xp:
                xT = xp.tile([128, 8 * SEQ], MMDT, tag="xT")
                wqT = xp.tile([128, 8 * DQ], MMDT, tag="wqT")
                wkT = xp.tile([128, 8 * DQ], MMDT, tag="wkT")
                wvT = xp.tile([128, 8 * DQ], MMDT, tag="wvT")

                xtv = XT.ap().rearrange("(j p) s -> p j s", p=128)
                for j in range(8):
                    nc.gpsimd.dma_start(
                        out=xT[:, SEQ * j:SEQ * (j + 1)], in_=xtv[:, j, :]
                    )
                for W, wT in ((WQT, wqT), (WKT, wkT), (WVT, wvT)):
                    wv_ = W.ap().rearrange("(j p) m -> p j m", p=128)
                    for g in range(2):
                        nc.gpsimd.dma_start(
                            out=wT[:, 1024 * g:1024 * (g + 1)], in_=wv_[:, 4 * g:4 * (g + 1), :]
                        )

                # qT / kT: lhsT (weight slice) reused across 4 n-chunks;
                # two [128,1024] psum tiles hold the 4 chunks
                for wT, qk, bt in ((wqT, qT, bq_t), (wkT, kT, bk_t)):
                    for mi in range(2):
                        pq = [psp.tile([128, 1024], F32, tag="score", name=f"pq{mi}_{u}")
                              for u in range(2)]
                        for j in range(8):
                            for n in range(4):
                                nc.tensor.matmul(
                                    pq[n // 2][:, 512 * (n % 2):512 * (n % 2 + 1)],
                                    wT[:, 256 * j + 128 * mi:256 * j + 128 * (mi + 1)],
                                    xT[:, SEQ * j + 512 * n:SEQ * j + 512 * (n + 1)],
                                    start=(j == 0),
                                    stop=(j == 7),
                                )
                        for u in range(2):
                            nc.scalar.activation(
                                qk[mi][:, 1024 * u:1024 * (u + 1)],
                                pq[u][:],
                                AF.Identity,
                                bias=bt[:, mi:mi + 1],
                                scale=1.0,
                            )

                # v projection into [V | 1] layout (ones cols via ACT casts)
                vview = vsb[:].rearrange("p (t h e) -> p t h e", t=16, h=HPC)
                for tt in range(16):
                    nc.scalar.copy(out=vview[:, tt, :, 64:65], in_=ones4[:].unsqueeze(-1))
                    pv = psp.tile([128, DQ], F32, tag="score", name=f"pv{tt}")
                    for j in range(8):
                        nc.tensor.matmul(
                            pv[:],
                            xT[:, SEQ * j + 128 * tt:SEQ * j + 128 * (tt + 1)],
                            wvT[:, 256 * j:256 * (j + 1)],
                            start=(j == 0),
                            stop=False,
                        )
                    nc.tensor.matmul(pv[:], ones1[:], bv_row[:], start=False, stop=True)
                    nc.scalar.copy(
                        out=vview[:, tt, :, 0:64],
                        in_=pv[:].rearrange("p (h d) -> p h d", h=HPC),
                    )

            # ---------- phase 2: attention (chunk-outer) + overlapped out-proj ----------
            with (
                tc.tile_pool(name="strips", bufs=20) as sp,
                tc.tile_pool(name="ostrips", bufs=2) as op2,
                tc.tile_pool(name="rp", bufs=1) as rp,
                tc.tile_pool(name="bp", bufs=2) as bp,
                tc.tile_pool(name="osb", bufs=1) as op,
            ):
                for ch in range(2):
                    sq0 = 1024 * ch
                    for h in range(HPC):
                        mi, po = h // 2, 64 * (h % 2)
                        att_h = ATT.ap()[h].rearrange("(t p) s -> p t s", p=128)
                        stiles = [sp.tile([128, 1024], MMDT, tag="strip",
                                          name=f"strip{h}_{ch}_{i}") for i in range(16)]
                        pc = psp.tile([65, 1024], F32, tag="ctx", name=f"pc{h}_{ch}")
                        for tt in range(16):
                            ps = psp.tile([128, 1024], F32, tag="score", name=f"ps{h}_{ch}_{tt}")
                            for n in range(2):
                                nc.tensor.matmul(
                                    ps[:, 512 * n:512 * (n + 1)],
                                    kT[mi][po:po + 64, 128 * tt:128 * (tt + 1)],
                                    qT[mi][po:po + 64, sq0 + 512 * n:sq0 + 512 * (n + 1)],
                                    start=True,
                                    stop=True,
                                )
                            nc.scalar.activation(
                                stiles[tt][:], ps[:], AF.Exp, scale=0.125,
                            )
                            for n in range(2):
                                nc.tensor.matmul(
                                    pc[:, 512 * n:512 * (n + 1)],
                                    vview[:, tt, h, :],
                                    stiles[tt][:, 512 * n:512 * (n + 1)],
                                    start=(tt == 0),
                                    stop=(tt == 15),
                                )
                        # --- reciprocal of row-sums on 128 lanes via PE transposes ---
                        zrow = rp.tile([1, 1024], F32, tag="zrow", name=f"zr{h}_{ch}")
                        nc.scalar.copy(out=zrow[:], in_=pc[64:65, :])
                        pzt = psp.tile([128, 1024], F32, tag="bcast", name=f"pzt{h}_{ch}")
                        for g in range(8):
                            nc.tensor.transpose(
                                pzt[:, g:g + 1], zrow[0:1, 128 * g:128 * (g + 1)],
                                ones1[0:1, 0:1],
                            )
                        rt = rp.tile([128, 8], F32, tag="rt", name=f"rt{h}_{ch}")
                        nc.vector.reciprocal(rt[:], pzt[:, 0:8])
                        prr = psp.tile([128, 1024], F32, tag="bcast", name=f"prr{h}_{ch}")
                        for g in range(8):
                            nc.tensor.transpose(
                                prr[0:1, 128 * g:128 * (g + 1)], rt[:, g:g + 1], ident[:]
                            )
                        rrow = rp.tile([1, 1024], F32, tag="zrow", name=f"rr{h}_{ch}")
                        nc.scalar.copy(out=rrow[:], in_=prr[0:1, :])
                        pb = psp.tile([128, 1024], F32, tag="bcast", name=f"pb{h}_{ch}")
                        for n in range(2):
                            nc.tensor.matmul(
                                pb[:, 512 * n:512 * (n + 1)],
                                ones1[:],
                                rrow[0:1, 512 * n:512 * (n + 1)],
                                start=True,
                                stop=True,
                            )
                        bcast = bp.tile([128, 1024], F32, tag="bcast_sb", name=f"bc{h}_{ch}")
                        nc.vector.tensor_copy(out=bcast[:], in_=pb[:])
                        # normalize A^T into f32 out-strips + DMA (DVE + GPSIMD split)
                        for tt in range(16):
                            ostrip = op2.tile([128, 1024], F32, tag="ostrip",
                                              name=f"os{h}_{ch}_{tt}")
                            eng = nc.gpsimd if tt < _GPS_TILES else nc.vector
                            eng.tensor_mul(
                                ostrip[:],
                                stiles[tt][:].bitcast(F32),
                                bcast[:],
                            )
                            nc.sync.dma_start(
                                out=att_h[:, tt, sq0:sq0 + 1024],
                                in_=ostrip[:],
                            )
                        # ctx normalize -> ctxT (PSUM in0 + SBUF in1 is legal)
                        nc.vector.tensor_mul(
                            ctxT[mi][po:po + 64, sq0:sq0 + 1024], pc[0:64, :], bcast[0:64, :]
                        )
                    # out-projection for this chunk's columns (overlaps next chunk)
                    for dt_ in range(8):
                        po_ = psp.tile([128, 1024], F32, tag="bcast", name=f"po{ch}_{dt_}")
                        for it in range(2):
                            for nn in range(2):
                                nc.tensor.matmul(
                                    po_[:, 512 * nn:512 * (nn + 1)],
                                    woT[it][:, 128 * dt_:128 * (dt_ + 1)],
                                    ctxT[it][:, sq0 + 512 * nn:sq0 + 512 * (nn + 1)],
                                    start=(it == 0),
                                    stop=(it == 1),
                                )
                        osb = op.tile([128, 1024], F32, tag="outT", name=f"osb{ch}_{dt_}")
                        nc.scalar.copy(out=osb[:], in_=po_[:])
                        nc.sync.dma_start(
                            out=OUTT.ap()[128 * dt_:128 * (dt_ + 1), sq0:sq0 + 1024],
                            in_=osb[:],
                        )

    nc.finalize()
    return nc


def _get_nc():
    if "nc" not in _state:
        _state["nc"] = _build_nc()
    return _state["nc"]


def _shard_inputs(x, Wq, bq, Wk, bk, Wv, bv, Wo, bo):
    f = lambda a: np.ascontiguousarray(np.asarray(a, dtype=np.float32))
    x, Wq, bq, Wk, bk, Wv, bv, Wo, bo = map(f, (x, Wq, bq, Wk, bk, Wv, bv, Wo, bo))
    xT = [np.ascontiguousarray(x[b].T) for b in range(BSZ)]            # [E, S]
    WqT, WkT, WvT = Wq.T, Wk.T, Wv.T                                   # [E, DQall]
    WoT = np.ascontiguousarray(Wo.T)                                   # [DQall, E]
    in_maps = []
    for c in range(NCORES):
        b, hb = c // 4, c % 4
        sl = slice(DQ * hb, DQ * (hb + 1))
        in_maps.append({
            "xt": xT[b],
            "wqt": np.ascontiguousarray(WqT[:, sl]), "bq": np.ascontiguousarray(bq[sl]),
            "wkt": np.ascontiguousarray(WkT[:, sl]), "bk": np.ascontiguousarray(bk[sl]),
            "wvt": np.ascontiguousarray(WvT[:, sl]), "bv": np.ascontiguousarray(bv[sl]),
            "wot": np.ascontiguousarray(WoT[sl, :]),
        })
    return in_maps, bo


def kernel(x, Wq, bq, Wk, bk, Wv, bv, Wo, bo):
    from concourse.bass_utils import run_bass_kernel_spmd

    nc = _get_nc()
    in_maps, bo_np = _shard_inputs(x, Wq, bq, Wk, bk, Wv, bv, Wo, bo)
    res = run_bass_kernel_spmd(nc, in_maps, core_ids=list(range(NCORES)))
    _state["last"] = res

    attnT = np.empty((BSZ, NHEADS, SEQ, SEQ), np.float32)
    out = np.zeros((BSZ, SEQ, EMBED), np.float32)
    for c in range(NCORES):
        b, hb = c // 4, c % 4
        r = res.results[c]
        attnT[b, HPC * hb:HPC * (hb + 1)] = r["attnT"]
        out[b] += r["outT"].T
    out += bo_np
    return out, attnT.swapaxes(2, 3)


# revision 20
# speedup vs baseline: 1.1178x; 1.0091x over previous
"""Multi-head attention Trainium2 kernel (8 NeuronCores, SPMD).

Sharding: core c handles batch b = c//4 and the 4 heads [4*(c%4), 4*(c%4)+4).
Attention is fully independent per (batch, head); the output projection is
computed as per-core partial sums over the core's 256 ctx columns and reduced
on the host (plus bias).

Host pre-transposes x and the weight blocks so the device loads operands
directly in contraction-on-partition layout (no on-chip transposes).

Per-core device program (matmuls contract along the partition dim):
  qT/kT = W @ xT + b                        [dq, sq]  (head dim on partitions)
  v     = x @ WvT + bv                      [t, dv]   (natural layout)
  per sq-chunk of 1024, per head:
    sT   = scores transposed [t, sq] tiles (lhsT = kT slice, rhs = qT slice)
    A    = exp(sT/8)  (ACT, PSUM->SBUF, rounded to matmul dtype)
    ctxT + row-sums via [V | 1] ones-column matmul, contract over t
    reciprocal via PE row<->column transposes (128-lane DVE recip)
    normalize with PE-broadcast reciprocal row (DVE+GPSIMD); DMA A^T strips
  outT  = WoT @ ctxT per chunk (overlapped with next chunk), host-reduced

The attention tensor is produced transposed ([h, t, sq] per core); kernel()
returns a numpy transposed view so no host transpose copy is needed.

Env knobs:
  ATTN_MM_DTYPE = f32r (default) | f32   -- matmul operand dtype
  ATTN_GPS_TILES = 5                     -- normalize tiles per chunk on GPSIMD
"""

import os

import numpy as np

EMBED = 1024
NHEADS = 16
DH = 64
BSZ = 2
SEQ = 2048
NCORES = 8
HPC = 4          # heads per core
DQ = HPC * DH    # 256 projection columns per core

_MM_MODE = os.environ.get("ATTN_MM_DTYPE", "f32r")
_GPS_TILES = int(os.environ.get("ATTN_GPS_TILES", "4"))

_state = {}


def _build_nc():
    import concourse.bacc as bacc
    import concourse.mybir as mybir
    from concourse import masks
    from concourse.tile import TileContext

    F32 = mybir.dt.float32
    MMDT = mybir.dt.float32r if _MM_MODE == "f32r" else F32
    AF = mybir.ActivationFunctionType

    nc = bacc.Bacc(trn_type="TRN2", target_bir_lowering=False)

    XT = nc.declare_dram_parameter("xt", [EMBED, SEQ], F32, isOutput=False)
    WQT = nc.declare_dram_parameter("wqt", [EMBED, DQ], F32, isOutput=False)
    BQ = nc.declare_dram_parameter("bq", [DQ], F32, isOutput=False)
    WKT = nc.declare_dram_parameter("wkt", [EMBED, DQ], F32, isOutput=False)
    BK = nc.declare_dram_parameter("bk", [DQ], F32, isOutput=False)
    WVT = nc.declare_dram_parameter("wvt", [EMBED, DQ], F32, isOutput=False)
    BV = nc.declare_dram_parameter("bv", [DQ], F32, isOutput=False)
    WOT = nc.declare_dram_parameter("wot", [DQ, EMBED], F32, isOutput=False)
    ATT = nc.declare_dram_parameter("attnT", [HPC, SEQ, SEQ], F32, isOutput=True)
    OUTT = nc.declare_dram_parameter("outT", [EMBED, SEQ], F32, isOutput=True)

    with TileContext(nc) as tc:
        with (
            tc.tile_pool(name="const", bufs=1) as cp,
            tc.tile_pool(name="persist", bufs=1) as pp,
            tc.tile_pool(name="psum", bufs=1, space="PSUM") as psp,
        ):
            ident = cp.tile([128, 128], F32, tag="ident")
            masks.make_identity(nc, ident[:])
            ones1 = cp.tile([1, 128], F32, tag="ones1")
            nc.vector.memset(ones1[:], 1.0)
            ones4 = cp.tile([128, 4], F32, tag="ones4")
            nc.vector.memset(ones4[:], 1.0)
            bq_t = cp.tile([128, 2], F32, tag="bqt")
            nc.sync.dma_start(out=bq_t[:], in_=BQ.ap().rearrange("(j p) -> p j", p=128))
            bk_t = cp.tile([128, 2], F32, tag="bkt")
            nc.sync.dma_start(out=bk_t[:], in_=BK.ap().rearrange("(j p) -> p j", p=128))
            bv_row = cp.tile([1, DQ], F32, tag="bvr")
            nc.sync.dma_start(out=bv_row[:], in_=BV.ap().unsqueeze(0))
            warm = cp.tile([1, 2], F32, tag="warm")
            nc.scalar.activation(warm[:], bq_t[0:1, 0:2], AF.Exp)

            qT = [pp.tile([128, SEQ], MMDT, tag=f"qT{i}", name=f"qT{i}") for i in range(2)]
            kT = [pp.tile([128, SEQ], MMDT, tag=f"kT{i}", name=f"kT{i}") for i in range(2)]
            vsb = pp.tile([128, 16 * HPC * 65], MMDT, tag="vsb")  # [t-tile][head][64+ones]
            ctxT = [pp.tile([128, SEQ], MMDT, tag=f"ctxT{i}", name=f"ctxT{i}") for i in range(2)]
            woT = [pp.tile([128, 1024], MMDT, tag=f"woT{i}", name=f"woT{i}") for i in range(2)]

            for j in range(2):
                nc.gpsimd.dma_start(out=woT[j][:], in_=WOT.ap()[128 * j:128 * (j + 1), :])

            # ---------- phase 1: projections ----------
            with tc.tile_pool(name="ph1sb", bufs=1) as xp:
                xT = xp.tile([128, 8 * SEQ], MMDT, tag="xT")
                wqT = xp.tile([128, 8 * DQ], MMDT, tag="wqT")
                wkT = xp.tile([128, 8 * DQ], MMDT, tag="wkT")
                wvT = xp.tile([128, 8 * DQ], MMDT, tag="wvT")

                xtv = XT.ap().rearrange("(j p) s -> p j s", p=128)
                for j in range(8):
                    nc.gpsimd.dma_start(
                        out=xT[:, SEQ * j:SEQ * (j + 1)], in_=xtv[:, j, :]
                    )
                for W, wT in ((WQT, wqT), (WKT, wkT), (WVT, wvT)):
                    wv_ = W.ap().rearrange("(j p) m -> p j m", p=128)
                    for g in range(2):
                        nc.gpsimd.dma_start(
                            out=wT[:, 1024 * g:1024 * (g + 1)], in_=wv_[:, 4 * g:4 * (g + 1), :]
                        )

                # qT / kT: lhsT (weight slice) reused across 4 n-chunks;
                # two [128,1024] psum tiles hold the 4 chunks
                for wT, qk, bt in ((wqT, qT, bq_t), (wkT, kT, bk_t)):
                    for mi in range(2):
                        pq = [psp.tile([128, 1024], F32, tag="score", name=f"pq{mi}_{u}")
                              for u in range(2)]
                        for j in range(8):
                            for n in range(4):
                                nc.tensor.matmul(
                                    pq[n // 2][:, 512 * (n % 2):512 * (n % 2 + 1)],
                                    wT[:, 256 * j + 128 * mi:256 * j + 128 * (mi + 1)],
                                    xT[:, SEQ * j + 512 * n:SEQ * j + 512 * (n + 1)],
                                    start=(j == 0),
                                    stop=(j == 7),
                                )
                        for u in range(2):
                            nc.scalar.activation(
                                qk[mi][:, 1024 * u:1024 * (u + 1)],
                                pq[u][:],
                                AF.Identity,
                                bias=bt[:, mi:mi + 1],
                                scale=1.0,
                            )

                # v projection into [V | 1] layout (ones cols via ACT casts)
                vview = vsb[:].rearrange("p (t h e) -> p t h e", t=16, h=HPC)
                for tt in range(16):
                    nc.scalar.copy(out=vview[:, tt, :, 64:65], in_=ones4[:].unsqueeze(-1))
                    pv = psp.tile([128, DQ], F32, tag="score", name=f"pv{tt}")
                    for j in range(8):
                        nc.tensor.matmul(
                            pv[:],
                            xT[:, SEQ * j + 128 * tt:SEQ * j + 128 * (tt + 1)],
                            wvT[:, 256 * j:256 * (j + 1)],
                            start=(j == 0),
                            stop=False,
                        )
                    nc.tensor.matmul(pv[:], ones1[:], bv_row[:], start=False, stop=True)
                    nc.scalar.copy(
                        out=vview[:, tt, :, 0:64],
                        in_=pv[:].rearrange("p (h d) -> p h d", h=HPC),
                    )

            # ---------- phase 2: attention (chunk-outer) + overlapped out-proj ----------
            with (
                tc.tile_pool(name="strips", bufs=20) as sp,
                tc.tile_pool(name="ostrips", bufs=2) as op2,
                tc.tile_pool(name="rp", bufs=1) as rp,
                tc.tile_pool(name="bp", bufs=2) as bp,
                tc.tile_pool(name="osb", bufs=1) as op,
            ):
                for ch in range(2):
                    sq0 = 1024 * ch
                    for h in range(HPC):
                        mi, po = h // 2, 64 * (h % 2)
                        att_h = ATT.ap()[h].rearrange("(t p) s -> p t s", p=128)
                        stiles = [sp.tile([128, 1024], MMDT, tag="strip",
                                          name=f"strip{h}_{ch}_{i}") for i in range(16)]
                        pc = psp.tile([65, 1024], F32, tag="ctx", name=f"pc{h}_{ch}")
                        for tt in range(16):
                            ps = psp.tile([128, 1024], F32, tag="score", name=f"ps{h}_{ch}_{tt}")
                            for n in range(2):
                                nc.tensor.matmul(
                                    ps[:, 512 * n:512 * (n + 1)],
                                    kT[mi][po:po + 64, 128 * tt:128 * (tt + 1)],
                                    qT[mi][po:po + 64, sq0 + 512 * n:sq0 + 512 * (n + 1)],
                                    start=True,
                                    stop=True,
                                )
                            nc.scalar.activation(
                                stiles[tt][:], ps[:], AF.Exp, scale=0.125,
                            )
                            for n in range(2):
                                nc.tensor.matmul(
                                    pc[:, 512 * n:512 * (n + 1)],
                                    vview[:, tt, h, :],
                                    stiles[tt][:, 512 * n:512 * (n + 1)],
                                    start=(tt == 0),
                                    stop=(tt == 15),
                                )
                        # --- reciprocal of row-sums on 128 lanes via PE transposes ---
                        zrow = rp.tile([1, 1024], F32, tag="zrow", name=f"zr{h}_{ch}")
                        nc.scalar.copy(out=zrow[:], in_=pc[64:65, :])
                        pzt = psp.tile([128, 1024], F32, tag="bcast", name=f"pzt{h}_{ch}")
                        for g in range(8):
                            nc.tensor.transpose(
                                pzt[:, g:g + 1], zrow[0:1, 128 * g:128 * (g + 1)],
                                ones1[0:1, 0:1],
                            )
                        rt = rp.tile([128, 8], F32, tag="rt", name=f"rt{h}_{ch}")
                        nc.vector.reciprocal(rt[:], pzt[:, 0:8])
                        prr = psp.tile([128, 1024], F32, tag="bcast", name=f"prr{h}_{ch}")
                        for g in range(8):
                            nc.tensor.transpose(
                                prr[0:1, 128 * g:128 * (g + 1)], rt[:, g:g + 1], ident[:]
                            )
                        rrow = rp.tile([1, 1024], F32, tag="zrow", name=f"rr{h}_{ch}")
                        nc.scalar.copy(out=rrow[:], in_=prr[0:1, :])
                        pb = psp.tile([128, 1024], F32, tag="bcast", name=f"pb{h}_{ch}")
                        for n in range(2):
                            nc.tensor.matmul(
                                pb[:, 512 * n:512 * (n + 1)],
                                ones1[:],
                                rrow[0:1, 512 * n:512 * (n + 1)],
                                start=True,
                                stop=True,
                            )
                        bcast = bp.tile([128, 1024], F32, tag="bcast_sb", name=f"bc{h}_{ch}")
                        nc.vector.tensor_copy(out=bcast[:], in_=pb[:])
                        # normalize A^T into f32 out-strips + DMA (DVE + GPSIMD split)
                        for tt in range(16):
                            ostrip = op2.tile([128, 1024], F32, tag="ostrip",
                                              name=f"os{h}_{ch}_{tt}")
                            eng = nc.gpsimd if tt < _GPS_TILES else nc.vector
                            eng.tensor_mul(
                                ostrip[:],
                                stiles[tt][:].bitcast(F32),
                                bcast[:],
                            )
                            nc.sync.dma_start(
                                out=att_h[:, tt, sq0:sq0 + 1024],
                                in_=ostrip[:],
                            )
                        # ctx normalize -> ctxT (PSUM in0 + SBUF in1 is legal)
                        nc.vector.tensor_mul(
                            ctxT[mi][po:po + 64, sq0:sq0 + 1024], pc[0:64, :], bcast[0:64, :]
                        )
                    # out-projection for this chunk's columns (overlaps next chunk)
                    for dt_ in range(8):
                        po_ = psp.tile([128, 1024], F32, tag="bcast", name=f"po{ch}_{dt_}")
                        for it in range(2):
                            for nn in range(2):
                                nc.tensor.matmul(
                                    po_[:, 512 * nn:512 * (nn + 1)],
                                    woT[it][:, 128 * dt_:128 * (dt_ + 1)],
                                    ctxT[it][:, sq0 + 512 * nn:sq0 + 512 * (nn + 1)],
                                    start=(it == 0),
                                    stop=(it == 1),
                                )
                        osb = op.tile([128, 1024], F32, tag="outT", name=f"osb{ch}_{dt_}")
                        nc.scalar.copy(out=osb[:], in_=po_[:])
                        nc.sync.dma_start(
                            out=OUTT.ap()[128 * dt_:128 * (dt_ + 1), sq0:sq0 + 1024],
                            in_=osb[:],
                        )

    nc.finalize()
    return nc


_LDW_OPT = os.environ.get("ATTN_LDW_OPT", "1") == "1"


def _patch_ldw_opt():
    if not _LDW_OPT or _state.get("ldw_patched"):
        return
    import concourse.bass_utils as bu

    orig = bu.run_command

    def run_command_ldwopt(argv, **kwargs):
        argv = ["--enable-ldw-opt=true" if a == "--enable-ldw-opt=false" else a
                for a in argv]
        return orig(argv, **kwargs)

    bu.run_command = run_command_ldwopt
    _state["ldw_patched"] = True


def _get_nc():
    if "nc" not in _state:
        _patch_ldw_opt()
        _state["nc"] = _build_nc()
    return _state["nc"]


def _shard_inputs(x, Wq, bq, Wk, bk, Wv, bv, Wo, bo):
    f = lambda a: np.ascontiguousarray(np.asarray(a, dtype=np.float32))
    x, Wq, bq, Wk, bk, Wv, bv, Wo, bo = map(f, (x, Wq, bq, Wk, bk, Wv, bv, Wo, bo))
    xT = [np.ascontiguousarray(x[b].T) for b in range(BSZ)]            # [E, S]
    WqT, WkT, WvT = Wq.T, Wk.T, Wv.T                                   # [E, DQall]
    WoT = np.ascontiguousarray(Wo.T)                                   # [DQall, E]
    in_maps = []
    for c in range(NCORES):
        b, hb = c // 4, c % 4
        sl = slice(DQ * hb, DQ * (hb + 1))
        in_maps.append({
            "xt": xT[b],
            "wqt": np.ascontiguousarray(WqT[:, sl]), "bq": np.ascontiguousarray(bq[sl]),
            "wkt": np.ascontiguousarray(WkT[:, sl]), "bk": np.ascontiguousarray(bk[sl]),
            "wvt": np.ascontiguousarray(WvT[:, sl]), "bv": np.ascontiguousarray(bv[sl]),
            "wot": np.ascontiguousarray(WoT[sl, :]),
        })
    return in_maps, bo


def kernel(x, Wq, bq, Wk, bk, Wv, bv, Wo, bo):
    from concourse.bass_utils import run_bass_kernel_spmd

    nc = _get_nc()
    in_maps, bo_np = _shard_inputs(x, Wq, bq, Wk, bk, Wv, bv, Wo, bo)
    res = run_bass_kernel_spmd(nc, in_maps, core_ids=list(range(NCORES)))
    _state["last"] = res

    attnT = np.empty((BSZ, NHEADS, SEQ, SEQ), np.float32)
    out = np.zeros((BSZ, SEQ, EMBED), np.float32)
    for c in range(NCORES):
        b, hb = c // 4, c % 4
        r = res.results[c]
        attnT[b, HPC * hb:HPC * (hb + 1)] = r["attnT"]
        out[b] += r["outT"].T
    out += bo_np
    return out, attnT.swapaxes(2, 3)


# revision 21
# speedup vs baseline: 1.2734x; 1.1392x over previous
"""Multi-head attention Trainium2 kernel (8 NeuronCores, SPMD).

Sharding: core c handles batch b = c//4 and the 4 heads [4*(c%4), 4*(c%4)+4).
Attention is fully independent per (batch, head); the output projection is
computed as per-core partial sums over the core's 256 ctx columns and reduced
on the host (plus bias).

Host pre-transposes x and the weight blocks so the device loads operands
directly in contraction-on-partition layout (no on-chip transposes).

Per-core device program (matmuls contract along the partition dim):
  qT/kT = W @ xT + b                        [dq, sq]  (head dim on partitions)
  v     = x @ WvT + bv                      [t, dv]   (natural layout)
  per sq-chunk of 1024, per head:
    sT   = scores transposed [t, sq] tiles (lhsT = kT slice, rhs = qT slice)
    A    = exp(sT/8)  (ACT, PSUM->SBUF, rounded to matmul dtype)
    ctxT + row-sums via [V | 1] ones-column matmul, contract over t
    reciprocal via PE row<->column transposes (128-lane DVE recip)
    normalize with PE-broadcast reciprocal row (DVE+GPSIMD); DMA A^T strips
  outT  = WoT @ ctxT per chunk (overlapped with next chunk), host-reduced

The attention tensor is produced transposed ([h, t, sq] per core); kernel()
returns a numpy transposed view so no host transpose copy is needed.

Env knobs:
  ATTN_MM_DTYPE = f32r (default) | f32   -- matmul operand dtype
  ATTN_GPS_TILES = 5                     -- normalize tiles per chunk on GPSIMD
"""

import os

import numpy as np

EMBED = 1024
NHEADS = 16
DH = 64
BSZ = 2
SEQ = 2048
NCORES = 8
HPC = 4          # heads per core
DQ = HPC * DH    # 256 projection columns per core

_MM_MODE = os.environ.get("ATTN_MM_DTYPE", "f32r")
_GPS_TILES = int(os.environ.get("ATTN_GPS_TILES", "4"))

_state = {}


def _build_nc():
    import concourse.bacc as bacc
    import concourse.mybir as mybir
    from concourse import masks
    from concourse.tile import TileContext

    F32 = mybir.dt.float32
    MMDT = mybir.dt.float32r if _MM_MODE == "f32r" else F32
    AF = mybir.ActivationFunctionType

    nc = bacc.Bacc(trn_type="TRN2", target_bir_lowering=False)

    XT = nc.declare_dram_parameter("xt", [EMBED, SEQ], F32, isOutput=False)
    WQT = nc.declare_dram_parameter("wqt", [EMBED, DQ], F32, isOutput=False)
    BQ = nc.declare_dram_parameter("bq", [DQ], F32, isOutput=False)
    WKT = nc.declare_dram_parameter("wkt", [EMBED, DQ], F32, isOutput=False)
    BK = nc.declare_dram_parameter("bk", [DQ], F32, isOutput=False)
    WVT = nc.declare_dram_parameter("wvt", [EMBED, DQ], F32, isOutput=False)
    BV = nc.declare_dram_parameter("bv", [DQ], F32, isOutput=False)
    WOT = nc.declare_dram_parameter("wot", [DQ, EMBED], F32, isOutput=False)
    ATT = nc.declare_dram_parameter("attnT", [HPC, SEQ, SEQ], F32, isOutput=True)
    OUTT = nc.declare_dram_parameter("outT", [EMBED, SEQ], F32, isOutput=True)

    with TileContext(nc) as tc:
        with (
            tc.tile_pool(name="const", bufs=1) as cp,
            tc.tile_pool(name="persist", bufs=1) as pp,
            tc.tile_pool(name="psum", bufs=1, space="PSUM") as psp,
        ):
            ident = cp.tile([128, 128], F32, tag="ident")
            masks.make_identity(nc, ident[:])
            ones1 = cp.tile([1, 128], F32, tag="ones1")
            nc.vector.memset(ones1[:], 1.0)
            ones4 = cp.tile([128, 4], F32, tag="ones4")
            nc.vector.memset(ones4[:], 1.0)
            bq_t = cp.tile([128, 2], F32, tag="bqt")
            nc.sync.dma_start(out=bq_t[:], in_=BQ.ap().rearrange("(j p) -> p j", p=128))
            bk_t = cp.tile([128, 2], F32, tag="bkt")
            nc.sync.dma_start(out=bk_t[:], in_=BK.ap().rearrange("(j p) -> p j", p=128))
            bv_row = cp.tile([1, DQ], F32, tag="bvr")
            nc.sync.dma_start(out=bv_row[:], in_=BV.ap().unsqueeze(0))
            warm = cp.tile([1, 2], F32, tag="warm")
            nc.scalar.activation(warm[:], bq_t[0:1, 0:2], AF.Exp)

            qT = [pp.tile([128, SEQ], MMDT, tag=f"qT{i}", name=f"qT{i}") for i in range(2)]
            kT = [pp.tile([128, SEQ], MMDT, tag=f"kT{i}", name=f"kT{i}") for i in range(2)]
            vsb = pp.tile([128, 16 * HPC * 65], MMDT, tag="vsb")  # [t-tile][head][64+ones]
            ctxT = [pp.tile([128, SEQ], MMDT, tag=f"ctxT{i}", name=f"ctxT{i}") for i in range(2)]
            woT = [pp.tile([128, 1024], MMDT, tag=f"woT{i}", name=f"woT{i}") for i in range(2)]

            for j in range(2):
                nc.gpsimd.dma_start(out=woT[j][:], in_=WOT.ap()[128 * j:128 * (j + 1), :])

            # ---------- phase 1: projections ----------
            with tc.tile_pool(name="ph1sb", bufs=1) as xp:
                xT = xp.tile([128, 8 * SEQ], MMDT, tag="xT")
                wqT = xp.tile([128, 8 * DQ], MMDT, tag="wqT")
                wkT = xp.tile([128, 8 * DQ], MMDT, tag="wkT")
                wvT = xp.tile([128, 8 * DQ], MMDT, tag="wvT")

                xtv = XT.ap().rearrange("(j p) s -> p j s", p=128)
                for j in range(8):
                    nc.gpsimd.dma_start(
                        out=xT[:, SEQ * j:SEQ * (j + 1)], in_=xtv[:, j, :]
                    )
                for W, wT in ((WQT, wqT), (WKT, wkT), (WVT, wvT)):
                    wv_ = W.ap().rearrange("(j p) m -> p j m", p=128)
                    for g in range(2):
                        nc.gpsimd.dma_start(
                            out=wT[:, 1024 * g:1024 * (g + 1)], in_=wv_[:, 4 * g:4 * (g + 1), :]
                        )

                # qT / kT: lhsT (weight slice) reused across 4 n-chunks;
                # two [128,1024] psum tiles hold the 4 chunks
                for wT, qk, bt in ((wqT, qT, bq_t), (wkT, kT, bk_t)):
                    for mi in range(2):
                        pq = [psp.tile([128, 1024], F32, tag="score", name=f"pq{mi}_{u}")
                              for u in range(2)]
                        for j in range(8):
                            for n in range(4):
                                nc.tensor.matmul(
                                    pq[n // 2][:, 512 * (n % 2):512 * (n % 2 + 1)],
                                    wT[:, 256 * j + 128 * mi:256 * j + 128 * (mi + 1)],
                                    xT[:, SEQ * j + 512 * n:SEQ * j + 512 * (n + 1)],
                                    start=(j == 0),
                                    stop=(j == 7),
                                )
                        for u in range(2):
                            nc.scalar.activation(
                                qk[mi][:, 1024 * u:1024 * (u + 1)],
                                pq[u][:],
                                AF.Identity,
                                bias=bt[:, mi:mi + 1],
                                scale=1.0,
                            )

                # v projection into [V | 1] layout (ones cols via ACT casts)
                vview = vsb[:].rearrange("p (t h e) -> p t h e", t=16, h=HPC)
                for tt in range(16):
                    nc.scalar.copy(out=vview[:, tt, :, 64:65], in_=ones4[:].unsqueeze(-1))
                    pv = psp.tile([128, DQ], F32, tag="score", name=f"pv{tt}")
                    for j in range(8):
                        nc.tensor.matmul(
                            pv[:],
                            xT[:, SEQ * j + 128 * tt:SEQ * j + 128 * (tt + 1)],
                            wvT[:, 256 * j:256 * (j + 1)],
                            start=(j == 0),
                            stop=False,
                        )
                    nc.tensor.matmul(pv[:], ones1[:], bv_row[:], start=False, stop=True)
                    nc.scalar.copy(
                        out=vview[:, tt, :, 0:64],
                        in_=pv[:].rearrange("p (h d) -> p h d", h=HPC),
                    )

            # ---------- phase 2: attention (chunk-outer) + overlapped out-proj ----------
            with (
                tc.tile_pool(name="strips", bufs=20) as sp,
                tc.tile_pool(name="ostrips", bufs=2) as op2,
                tc.tile_pool(name="rp", bufs=1) as rp,
                tc.tile_pool(name="bp", bufs=2) as bp,
                tc.tile_pool(name="osb", bufs=1) as op,
            ):
                for ch in range(2):
                    sq0 = 1024 * ch
                    for h in range(HPC):
                        mi, po = h // 2, 64 * (h % 2)
                        att_h = ATT.ap()[h].rearrange("(t p) s -> p t s", p=128)
                        stiles = [sp.tile([128, 1024], MMDT, tag="strip",
                                          name=f"strip{h}_{ch}_{i}") for i in range(16)]
                        pc = psp.tile([65, 1024], F32, tag="ctx", name=f"pc{h}_{ch}")
                        def emit_ctx(tt):
                            for n in range(2):
                                nc.tensor.matmul(
                                    pc[:, 512 * n:512 * (n + 1)],
                                    vview[:, tt, h, :],
                                    stiles[tt][:, 512 * n:512 * (n + 1)],
                                    start=(tt == 0),
                                    stop=(tt == 15),
                                )

                        for tt in range(16):
                            ps = psp.tile([128, 1024], F32, tag="score", name=f"ps{h}_{ch}_{tt}")
                            for n in range(2):
                                nc.tensor.matmul(
                                    ps[:, 512 * n:512 * (n + 1)],
                                    kT[mi][po:po + 64, 128 * tt:128 * (tt + 1)],
                                    qT[mi][po:po + 64, sq0 + 512 * n:sq0 + 512 * (n + 1)],
                                    start=True,
                                    stop=True,
                                )
                            nc.scalar.activation(
                                stiles[tt][:], ps[:], AF.Exp, scale=0.125,
                            )
                            if tt >= 1:
                                emit_ctx(tt - 1)
                        emit_ctx(15)
                        # --- reciprocal of row-sums on 128 lanes via PE transposes ---
                        zrow = rp.tile([1, 1024], F32, tag="zrow", name=f"zr{h}_{ch}")
                        nc.scalar.copy(out=zrow[:], in_=pc[64:65, :])
                        pzt = psp.tile([128, 1024], F32, tag="bcast", name=f"pzt{h}_{ch}")
                        for g in range(8):
                            nc.tensor.transpose(
                                pzt[:, g:g + 1], zrow[0:1, 128 * g:128 * (g + 1)],
                                ones1[0:1, 0:1],
                            )
                        rt = rp.tile([128, 8], F32, tag="rt", name=f"rt{h}_{ch}")
                        nc.vector.reciprocal(rt[:], pzt[:, 0:8])
                        prr = psp.tile([128, 1024], F32, tag="bcast", name=f"prr{h}_{ch}")
                        for g in range(8):
                            nc.tensor.transpose(
                                prr[0:1, 128 * g:128 * (g + 1)], rt[:, g:g + 1], ident[:]
                            )
                        rrow = rp.tile([1, 1024], F32, tag="zrow", name=f"rr{h}_{ch}")
                        nc.scalar.copy(out=rrow[:], in_=prr[0:1, :])
                        pb = psp.tile([128, 1024], F32, tag="bcast", name=f"pb{h}_{ch}")
                        for n in range(2):
                            nc.tensor.matmul(
                                pb[:, 512 * n:512 * (n + 1)],
                                ones1[:],
                                rrow[0:1, 512 * n:512 * (n + 1)],
                                start=True,
                                stop=True,
                            )
                        bcast = bp.tile([128, 1024], F32, tag="bcast_sb", name=f"bc{h}_{ch}")
                        nc.vector.tensor_copy(out=bcast[:], in_=pb[:])
                        # normalize A^T into f32 out-strips + DMA (DVE + GPSIMD split)
                        for tt in range(16):
                            ostrip = op2.tile([128, 1024], F32, tag="ostrip",
                                              name=f"os{h}_{ch}_{tt}")
                            eng = nc.gpsimd if tt < _GPS_TILES else nc.vector
                            eng.tensor_mul(
                                ostrip[:],
                                stiles[tt][:].bitcast(F32),
                                bcast[:],
                            )
                            nc.sync.dma_start(
                                out=att_h[:, tt, sq0:sq0 + 1024],
                                in_=ostrip[:],
                            )
                        # ctx normalize -> ctxT (PSUM in0 + SBUF in1 is legal)
                        nc.vector.tensor_mul(
                            ctxT[mi][po:po + 64, sq0:sq0 + 1024], pc[0:64, :], bcast[0:64, :]
                        )
                    # out-projection for this chunk's columns (overlaps next chunk)
                    for dt_ in range(8):
                        po_ = psp.tile([128, 1024], F32, tag="bcast", name=f"po{ch}_{dt_}")
                        for it in range(2):
                            for nn in range(2):
                                nc.tensor.matmul(
                                    po_[:, 512 * nn:512 * (nn + 1)],
                                    woT[it][:, 128 * dt_:128 * (dt_ + 1)],
                                    ctxT[it][:, sq0 + 512 * nn:sq0 + 512 * (nn + 1)],
                                    start=(it == 0),
                                    stop=(it == 1),
                                )
                        osb = op.tile([128, 1024], F32, tag="outT", name=f"osb{ch}_{dt_}")
                        nc.scalar.copy(out=osb[:], in_=po_[:])
                        nc.sync.dma_start(
                            out=OUTT.ap()[128 * dt_:128 * (dt_ + 1), sq0:sq0 + 1024],
                            in_=osb[:],
                        )

    nc.finalize()
    return nc


_LDW_OPT = os.environ.get("ATTN_LDW_OPT", "1") == "1"


def _patch_ldw_opt():
    if not _LDW_OPT or _state.get("ldw_patched"):
        return
    import concourse.bass_utils as bu

    orig = bu.run_command

    def run_command_ldwopt(argv, **kwargs):
        argv = ["--enable-ldw-opt=true" if a == "--enable-ldw-opt=false" else a
                for a in argv]
        return orig(argv, **kwargs)

    bu.run_command = run_command_ldwopt
    _state["ldw_patched"] = True


def _get_nc():
    if "nc" not in _state:
        _patch_ldw_opt()
        _state["nc"] = _build_nc()
    return _state["nc"]


def _shard_inputs(x, Wq, bq, Wk, bk, Wv, bv, Wo, bo):
    f = lambda a: np.ascontiguousarray(np.asarray(a, dtype=np.float32))
    x, Wq, bq, Wk, bk, Wv, bv, Wo, bo = map(f, (x, Wq, bq, Wk, bk, Wv, bv, Wo, bo))
    xT = [np.ascontiguousarray(x[b].T) for b in range(BSZ)]            # [E, S]
    WqT, WkT, WvT = Wq.T, Wk.T, Wv.T                                   # [E, DQall]
    WoT = np.ascontiguousarray(Wo.T)                                   # [DQall, E]
    in_maps = []
    for c in range(NCORES):
        b, hb = c // 4, c % 4
        sl = slice(DQ * hb, DQ * (hb + 1))
        in_maps.append({
            "xt": xT[b],
            "wqt": np.ascontiguousarray(WqT[:, sl]), "bq": np.ascontiguousarray(bq[sl]),
            "wkt": np.ascontiguousarray(WkT[:, sl]), "bk": np.ascontiguousarray(bk[sl]),
            "wvt": np.ascontiguousarray(WvT[:, sl]), "bv": np.ascontiguousarray(bv[sl]),
            "wot": np.ascontiguousarray(WoT[sl, :]),
        })
    return in_maps, bo


def kernel(x, Wq, bq, Wk, bk, Wv, bv, Wo, bo):
    from concourse.bass_utils import run_bass_kernel_spmd

    nc = _get_nc()
    in_maps, bo_np = _shard_inputs(x, Wq, bq, Wk, bk, Wv, bv, Wo, bo)
    res = run_bass_kernel_spmd(nc, in_maps, core_ids=list(range(NCORES)))
    _state["last"] = res

    attnT = np.empty((BSZ, NHEADS, SEQ, SEQ), np.float32)
    out = np.zeros((BSZ, SEQ, EMBED), np.float32)
    for c in range(NCORES):
        b, hb = c // 4, c % 4
        r = res.results[c]
        attnT[b, HPC * hb:HPC * (hb + 1)] = r["attnT"]
        out[b] += r["outT"].T
    out += bo_np
    return out, attnT.swapaxes(2, 3)
